# revision 1
# baseline (speedup 1.0000x reference)
"""Distributed Bass kernel for nn_Attention_57612691309274 on 8 TRN2 NeuronCores.

Reference computes, for x [B=2, S=2048, D=1024], H=16 heads, Dh=64:
  q/k/v = einsum('bsd,hde->bshe', x, W) + b, scaled by 1/sqrt(D)
  scores = q@k^T / sqrt(Dh), causal mask, softmax
  out = ((softmax @ v) @ W_O) * 1/sqrt(Dh) + b_O

Sharding: core c => batch b = c//4, head-group hg = c%4 (heads 4hg..4hg+3).
Each core projects q/k/v for its 4 heads over its batch and runs causal
attention in a [k, q]-transposed score layout (softmax needs no partition
reduction; umup scaling keeps scores ~N(0,1) so no max subtraction).

Key structure:
 - AV matmul stationary per head is [ones(64 cols) | v(64 cols)]: PSUM rows
   0-63 hold the softmax denominator replicated across 64 partitions BY THE
   PE (free: matmul cost is moving-column bound), rows 64-127 the
   unnormalized z. Normalization = reciprocal_approx_fast on rows 0-63
   (base partition 0 — the custom DVE op reads its input at the OUTPUT's
   base partition, so in/out bases must match) + one multiply reading the
   PSUM z rows at base 64 (PSUM operands carry their own base field; two
   SBUF inputs would have to share one). No DRAM round trips and the exact
   reciprocal (6 cyc/elem) is avoided.
 - Per head pair, z accumulates in two single-bank PSUM tiles (one per
   head) so the next pair's accumulation only waits on one head's
   normalize, not the whole chain.
 - Head pairs (even head on PE rows 0-63, odd on 64-127) issue score
   matmuls to disjoint row-groups that run concurrently and share one exp.
   The two heads' scores always land in separate PSUM banks (concurrent
   row-group matmuls must not share a bank).
 - Emission is software-pipelined and interleaved: score(t+1) is emitted
   before AV(t), and qkv-projection / out-projection "filler" chunks are
   spread between attention tiles so the PE never idles while exp runs and
   the scalar engine is fed as early as possible. Filler chunks are
   self-contained (a PSUM tile held across a filler gap deadlocks the
   round-robin pool).
 - Input loads are phased: the first q-projection's operands (wq + xT
   block 0, chunked per 128 columns in consumption order) go out first
   across the sync/gpsimd/scalar queues; everything else queues behind.
 - Each query block's partial out-projection (local 256 head-dims) is
   combined with a chunked ReduceScatter(add) over the 4 cores of the
   batch group, issued as blocks complete. The last 512-row block is split
   into two 256-row halves so the final RS overlaps remaining attention;
   the halves also pair two 256-wide k-tiles per exp call to cut ACTIVATE
   overhead where the scalar engine is tightest.
 - Core at group position g receives, per full block j, query rows
   512j+128g..+128, and for the block-3 halves rows 1536+64g and 1792+64g.

All umup scale factors are folded on the host into W_Q (1/8192) and W_O
(1/256). x is fed pre-transposed and pre-cast to bf16.
"""

import os
import sys

if "/opt/trn_rl_repo" not in sys.path:
    sys.path.insert(0, "/opt/trn_rl_repo")

import numpy as np
import ml_dtypes

import concourse.bass as bass
import concourse.tile as tile
from concourse import bacc, mybir

BF16 = np.dtype(ml_dtypes.bfloat16)
F32 = np.float32

B, S, D, H, DH = 2, 2048, 1024, 16, 64
HC = 4            # heads per core
E = HC * DH       # 256 head-dim columns per core
N_CORES = 8
CORE_IDS = list(range(N_CORES))
QB = 512          # full query block
P = 128

_NC_CACHE = {}


def build_kernel():
    nc = bacc.Bacc("TRN2", target_bir_lowering=False, debug=False,
                   num_devices=N_CORES)
    dt = mybir.dt

    # ---- external I/O (per-core shards fed from host) ----
    xT_d = nc.dram_tensor("xT", [D, S], dt.bfloat16, kind="ExternalInput")
    wq_d = nc.dram_tensor("wq", [D, E], dt.bfloat16, kind="ExternalInput")
    wk_d = nc.dram_tensor("wk", [D, E], dt.bfloat16, kind="ExternalInput")
    wv_d = nc.dram_tensor("wv", [D, E], dt.bfloat16, kind="ExternalInput")
    wo_d = nc.dram_tensor("wo", [E, D], dt.bfloat16, kind="ExternalInput")
    bq_d = nc.dram_tensor("bq", [P, 2], dt.float32, kind="ExternalInput")
    bk_d = nc.dram_tensor("bk", [P, 2], dt.float32, kind="ExternalInput")
    bv_d = nc.dram_tensor("bv", [P, E], dt.float32, kind="ExternalInput")
    bo_d = nc.dram_tensor("bo", [1, D], dt.float32, kind="ExternalInput")
    tri_d = nc.dram_tensor("tri", [P, P], dt.bfloat16, kind="ExternalInput")
    # out rows: [0:128)=blk0, [128:256)=blk1, [256:384)=blk2,
    # [384:448)=blk3a, [448:512)=blk3b   (per-group-member slices)
    out_d = nc.dram_tensor("out", [QB, D], dt.bfloat16, kind="ExternalOutput")

    # ---- internal DRAM ----
    rs_in = nc.dram_tensor("rs_in", [S, D], dt.bfloat16)
    rs_out = nc.dram_tensor("rs_out", [QB, D], dt.bfloat16)

    groups = [[0, 1, 2, 3], [4, 5, 6, 7]]
    Exp = mybir.ActivationFunctionType.Exp
    ADD = mybir.AluOpType.add
    MUL = mybir.AluOpType.mult

    with tile.TileContext(nc) as tc:
        with (
            tc.tile_pool(name="persist", bufs=1) as pp,
            tc.tile_pool(name="etile", bufs=8) as ep,
            tc.tile_pool(name="obuf", bufs=2) as op_,
            tc.tile_pool(name="rcp", bufs=2) as rp,
            tc.tile_pool(name="sc", bufs=3, space="PSUM") as scp,
            tc.tile_pool(name="zp", bufs=2, space="PSUM") as zpp,
        ):
            # ---------- staged input loads ----------
            # Phase 1 (everything the first ~25us of compute needs, and
            # nothing else, so the shared DMA bandwidth all goes to it):
            # wq + xT block 0 on sync, wk/wv on scalar. Later blocks and
            # constants are emitted behind them.
            xT = pp.tile([P, 8, S], dt.bfloat16, tag="xT")
            xT_v = xT_d.ap().rearrange("(o p) f -> p o f", p=P)
            wq = pp.tile([P, 8, E], dt.bfloat16, tag="wq")
            wq_v = wq_d.ap().rearrange("(o p) f -> p o f", p=P)
            wk = pp.tile([P, 8, E], dt.bfloat16, tag="wk")
            wk_v = wk_d.ap().rearrange("(o p) f -> p o f", p=P)
            # Tiny consts go FIRST: DMA-completion semaphores are a small
            # reused pool assigned in emission order, and a consumer waits on
            # its dep-DMA's (sem, threshold) — if a const shares a sem slot
            # behind a 1MB transfer, its consumer falsely waits on that
            # transfer (measured: 10us vector stall -> PE idle -> HAM cold).
            bq = pp.tile([P, 2], dt.float32, tag="bq")
            nc.sync.dma_start(bq[:], bq_d.ap()[:])
            bk = pp.tile([P, 2], dt.float32, tag="bk")
            nc.sync.dma_start(bk[:], bk_d.ap()[:])
            tri = pp.tile([P, P], dt.bfloat16, tag="tri")
            nc.sync.dma_start(tri[:], tri_d.ap()[:])
            bv = pp.tile([P, E], dt.float32, tag="bv")
            nc.gpsimd.dma_start(bv[:], bv_d.ap()[:])
            # chunked in the order the first q-projection consumes them, and
            # with the t=0 matmul's two operands (wq chunk, xT chunk) leading
            # DIFFERENT queues so they transfer in parallel: the first matmul
            # waits on ~200KB landing concurrently, not serially. (Only
            # sync/gpsimd/scalar queues can issue DMAs.)
            for t in range(8):
                nc.gpsimd.dma_start(xT[:, t:t + 1, 0:QB], xT_v[:, t:t + 1, 0:QB])
                nc.sync.dma_start(wq[:, t:t + 1], wq_v[:, t:t + 1])
                nc.scalar.dma_start(wk[:, t:t + 1], wk_v[:, t:t + 1])
            wv = pp.tile([P, 8, E], dt.bfloat16, tag="wv")
            nc.sync.dma_start(wv[:], wv_d.ap().rearrange("(o p) f -> p o f", p=P))
            # Phase 2: remaining xT blocks + wo. Block 1 rides the scalar
            # queue — it carries only the 0.5MB of wk chunks, so b1 lands
            # ~22-28us, before j=0's qkv(1) filler matmuls need it. (Putting
            # b1 on gpsimd behind the 1MB block-0 load was measured worse:
            # it arrives ~39us and stalls the tensor stream at the fillers.)
            nc.scalar.dma_start(xT[:, :, QB:2 * QB], xT_v[:, :, QB:2 * QB])
            wo = pp.tile([P, 2, D], dt.bfloat16, tag="wo")
            nc.gpsimd.dma_start(wo[:], wo_d.ap().rearrange("(o p) f -> p o f", p=P))
            # bo (512KB broadcast write) is only needed at ~55us; late
            # semaphore sharing is harmless by then
            bo = pp.tile([P, D], dt.float32, tag="bo")
            nc.gpsimd.dma_start(bo[:], bo_d.ap()[0:1, :].to_broadcast([P, D]))
            nc.sync.dma_start(xT[:, :, 2 * QB:3 * QB], xT_v[:, :, 2 * QB:3 * QB])
            nc.gpsimd.dma_start(xT[:, :, 3 * QB:4 * QB], xT_v[:, :, 3 * QB:4 * QB])

            qT = pp.tile([P, 2, S], dt.bfloat16, tag="qT")
            kT = pp.tile([P, 2, S], dt.bfloat16, tag="kT")
            # per head h: cols [128h, 128h+64) = 1.0, [128h+64, 128h+128) = v
            # (AV output rows 0-63 = softmax denominator replicated by the PE
            # at base partition 0 — required by reciprocal_approx_fast —
            # rows 64-127 = unnormalized z)
            vsb = pp.tile([P, 16, HC * P], dt.bfloat16, tag="vsb")
            nc.gpsimd.memset(
                vsb.rearrange("p t (h c) -> p t h c", h=HC)[:, :, :, 0:DH], 1.0)
            # z^T staging, laid out [p, e-tile(2), q-block(4), 512]
            zT = pp.tile([P, 2, 4, QB], dt.bfloat16, tag="zT")

            # ---------- filler chunk emitters (qkv projections / outproj) ----
            def emit_qk_half(jb, which, m):
                """One m-half (8 MMs + bias add) of a q/k projection for
                block jb. Self-contained: the PSUM tile is released before
                the next chunk (holding it across filler gaps deadlocks the
                round-robin pool)."""
                w_sb, b_sb, dst = {
                    "q": (wq, bq, qT), "k": (wk, bk, kT)}[which]
                ps = scp.tile([P, 2, QB], dt.float32, tag="sc", name="qkps")
                for t in range(8):
                    nc.tensor.matmul(
                        ps[:, m, :],
                        lhsT=w_sb[:, t, P * m:P * (m + 1)],
                        rhs=xT[:, t, QB * jb:QB * (jb + 1)],
                        start=(t == 0), stop=(t == 7),
                    )
                nc.vector.tensor_tensor(
                    out=dst[:, m, QB * jb:QB * (jb + 1)],
                    in0=ps[:, m, :],
                    in1=b_sb[:, m, None].to_broadcast([P, QB]),
                    op=ADD,
                )

            def emit_v_tile(jt):
                """v projection for one 128-row tile of the sequence."""
                ps = scp.tile([P, 2, QB], dt.float32, tag="sc")
                psv = ps[:, 0, :E]
                for t in range(8):
                    nc.tensor.matmul(
                        psv,
                        lhsT=xT[:, t, P * jt:P * (jt + 1)],
                        rhs=wv[:, t, :],
                        start=(t == 0), stop=(t == 7),
                    )
                nc.vector.tensor_tensor(
                    out=vsb[:, jt].rearrange(
                        "p (h c) -> p h c", h=HC)[:, :, DH:P],
                    in0=psv.rearrange("p (h e) -> p h e", h=HC),
                    in1=bv.rearrange("p (h e) -> p h e", h=HC),
                    op=ADD,
                )

            ob_live = {}

            def emit_op_qq(j, zq0, qq, row0, orow0, nrows, rs_after,
                           copy_after=None):
                """One 128-query-row tile of the partial out-projection for
                block j (zq0 = column offset into zT's q axis). After the
                second qq of a 256-row pair, DMA-stage to rs_in; after the
                last, trigger the ReduceScatter."""
                qt = zq0 // P + qq
                pair = qq // 2
                if qq % 2 == 0:
                    ob_live[j] = op_.tile([P, 2, 2, QB], dt.bfloat16,
                                          tag="opb", name="ob")
                ob = ob_live[j]
                ps = scp.tile([P, 2, QB], dt.float32, tag="sc")
                for nb in range(2):
                    for t2 in range(2):
                        nc.tensor.matmul(
                            ps[:, nb, :],
                            lhsT=zT[:, t2, j, P * qt:P * (qt + 1)],
                            rhs=wo[:, t2, QB * nb:QB * (nb + 1)],
                            start=(t2 == 0), stop=(t2 == 1),
                        )
                # bo holds b_O/4: each group member adds a quarter, so the
                # ReduceScatter sum carries the full bias.
                nc.vector.tensor_tensor(
                    out=ob[:, qq % 2], in0=ps[:],
                    in1=bo.rearrange("p (n f) -> p n f", n=2), op=ADD)
                # stage each 128-row tile as soon as its bias add lands, so
                # the final tile's staging is all that precedes the RS
                r0 = row0 + P * qq
                nc.sync.dma_start(
                    rs_in.ap()[r0:r0 + P, :].rearrange(
                        "p (n f) -> p n f", n=2),
                    ob[:, qq % 2])
                if qq % 2 == 1:
                    del ob_live[j]
                if rs_after:
                    nc.gpsimd.collective_compute(
                        "ReduceScatter",
                        ADD,
                        replica_groups=groups,
                        ins=[rs_in.ap()[row0:row0 + nrows, :].opt()],
                        outs=[rs_out.ap()[orow0:orow0 + nrows // 4, :].opt()],
                    )

            # ---------- attention stream ----------
            def attention_stream(j, q0, nq, n_kt, zq0, fillers, sched=None):
                """Causal attention for query rows [q0, q0+nq) of block j,
                over n_kt 128-row k-tiles, for both head pairs. Emission is
                software-pipelined (score(t+1) before AV(t)) and filler
                chunks are spread across AV points; any not reached are
                flushed at the end of the stream."""
                n_f = len(fillers)
                if sched is None:
                    sched = [(i + 1) * (2 * n_kt) / (n_f + 1)
                             for i in range(n_f)]
                fq = list(fillers)
                av_i = 0

                def maybe_fill():
                    nonlocal av_i
                    av_i += 1
                    while fq and sched[n_f - len(fq)] <= av_i:
                        fq.pop(0)()

                def emit_mask_exp(t, psc3, et3):
                    """exp + causal masking for one k-tile given [2, nq]
                    views of its score PSUM and exp output."""
                    s = t - q0 // P
                    if s < 0:      # fully unmasked tile
                        nc.scalar.activation(et3[:, :, :nq], psc3[:, :, :nq],
                                             Exp)
                    else:          # diagonal-crossing tile
                        if s > 0:
                            nc.vector.memset(et3[:, :, 0:P * s], 0.0)
                        nc.scalar.activation(
                            et3[:, :, P * s:nq], psc3[:, :, P * s:nq], Exp)
                        nc.vector.tensor_tensor(
                            out=et3[:, :, P * s:P * (s + 1)],
                            in0=et3[:, :, P * s:P * (s + 1)],
                            in1=tri[:, None, :].to_broadcast([P, 2, P]),
                            op=MUL,
                        )

                for hp in range(2):
                    # one single-bank PSUM tile per head (g): the next hp's
                    # g-accumulation only waits on THIS g's normalize, and
                    # the two normalizes pipeline.
                    pza = zpp.tile([P, QB], dt.float32, tag="z", name="pza")
                    pzb = zpp.tile([P, QB], dt.float32, tag="z", name="pzb")
                    pz = (pza, pzb)
                    prev = None

                    def emit_av(t, et, tp=None):
                        for g in range(2):
                            h = 2 * hp + g
                            rhs = (et[:, g, :nq] if tp is None
                                   else et[:, g, tp, :nq])
                            nc.tensor.matmul(
                                pz[g][:, :nq],
                                lhsT=vsb[:, t, P * h:P * (h + 1)],
                                rhs=rhs,
                                start=(t == 0), stop=(t == n_kt - 1),
                                skip_group_check=True,
                            )

                    def emit_score(t, out2):
                        for g in range(2):
                            b0 = 64 * g
                            nc.tensor.matmul(
                                out2[:, g, :nq],
                                lhsT=kT[b0:b0 + 64, hp, P * t:P * (t + 1)],
                                rhs=qT[b0:b0 + 64, hp, q0:q0 + nq],
                                start=True, stop=True,
                                tile_position=(b0, 0),
                            )

                    if nq == QB:
                        for t in range(n_kt):
                            psc = scp.tile([P, 2, QB], dt.float32, tag="sc")
                            emit_score(t, psc)
                            et = ep.tile([P, 2, QB], dt.bfloat16, tag="et")
                            emit_mask_exp(t, psc, et)
                            if prev is not None:
                                emit_av(*prev)
                                maybe_fill()
                            prev = (t, et)
                        emit_av(*prev)
                        maybe_fill()
                    else:
                        # 256-wide half blocks: two k-tiles share one PSUM
                        # tile and (when both are unmasked) one exp call.
                        # Layout [g, tp, 256] keeps each head pair's scores
                        # in its own bank (the two row-group matmuls run
                        # concurrently and must not share a PSUM bank).
                        for pr in range(n_kt // 2):
                            t0, t1 = 2 * pr, 2 * pr + 1
                            psc = scp.tile([P, 2, 2, QB // 2], dt.float32,
                                           tag="sc", name="pscp")
                            emit_score(t0, psc[:, :, 0])
                            emit_score(t1, psc[:, :, 1])
                            et = ep.tile([P, 2, 2, QB // 2], dt.bfloat16,
                                         tag="et", name="etp")
                            if t1 - q0 // P < 0:   # both unmasked: one exp
                                nc.scalar.activation(et[:], psc[:], Exp)
                            else:
                                emit_mask_exp(t0, psc[:, :, 0], et[:, :, 0])
                                emit_mask_exp(t1, psc[:, :, 1], et[:, :, 1])
                            if prev is not None:
                                for pt in prev[0]:
                                    emit_av(pt, prev[1], tp=pt % 2)
                                maybe_fill()
                                maybe_fill()
                            prev = ((t0, t1), et)
                        for pt in prev[0]:
                            emit_av(pt, prev[1], tp=pt % 2)
                        maybe_fill()
                        maybe_fill()
                    if hp == 1:
                        while fq:       # safety net; sched places everything
                            fq.pop(0)()
                    # normalize: rows 0-63 of each pz hold the denominator
                    # (replicated by the PE), rows 64-127 the unnormalized z;
                    # fast reciprocal + multiply straight from PSUM, per head
                    # so the banks free one by one.
                    rcp = rp.tile([DH, 2, QB], dt.float32, tag="rcp")
                    for g in range(2):
                        nc.vector.reciprocal_approx_fast(
                            out=rcp[:, g, :nq], in_=pz[g][0:DH, :nq])
                        nc.vector.tensor_tensor(
                            out=zT[64 * g:64 * (g + 1), hp, j,
                                   zq0:zq0 + nq],
                            in0=pz[g][DH:P, :nq],
                            in1=rcp[:, g, :nq],
                            op=MUL,
                        )

            # ---------- main schedule ----------
            # qkv block 0 up front (nothing to overlap with yet). Block 0 is
            # DMA-paced, so all four accumulation chains (q-m0/q-m1/k-m0/
            # k-m1, four separate PSUM banks) interleave per k-chunk: each
            # arriving 128KB chunk feeds 4 matmuls instead of 1, finishing
            # q(0)+k(0) ~5us earlier than serial m-halves.
            # (A PE warm-up via dummy matmuls was tried here and reverted:
            # when the firmware thermal throttle clamps the clock arbiter,
            # activity cannot lift the HAM gate and the dummies only add
            # ~6us of real PE work.)
            ps_q = scp.tile([P, 2, QB], dt.float32, tag="sc", name="ps_q")
            ps_k = scp.tile([P, 2, QB], dt.float32, tag="sc", name="ps_k")
            for t in range(8):
                for w_sb, ps in ((wq, ps_q), (wk, ps_k)):
                    for m in range(2):
                        nc.tensor.matmul(
                            ps[:, m, :],
                            lhsT=w_sb[:, t, P * m:P * (m + 1)],
                            rhs=xT[:, t, 0:QB],
                            start=(t == 0), stop=(t == 7),
                            skip_group_check=True,
                        )
            for b_sb, ps, dst in ((bq, ps_q, qT), (bk, ps_k, kT)):
                for m in range(2):
                    nc.vector.tensor_tensor(
                        out=dst[:, m, 0:QB],
                        in0=ps[:, m, :],
                        in1=b_sb[:, m, None].to_broadcast([P, QB]),
                        op=ADD,
                    )
            for jt in range(4):
                emit_v_tile(jt)

            def qkv_chunks(jb):
                return [
                    lambda jb=jb: emit_qk_half(jb, "q", 0),
                    lambda jb=jb: emit_qk_half(jb, "q", 1),
                    lambda jb=jb: emit_qk_half(jb, "k", 0),
                    lambda jb=jb: emit_qk_half(jb, "k", 1),
                ] + [lambda jt=jt: emit_v_tile(jt) for jt in range(4 * jb,
                                                                  4 * jb + 4)]

            def op_chunks(j, zq0, row0, orow0, nrows, copy_after=None):
                n_qq = nrows // P
                return [
                    lambda qq=qq: emit_op_qq(j, zq0, qq, row0, orow0, nrows,
                                             rs_after=(qq == n_qq - 1),
                                             copy_after=copy_after)
                    for qq in range(n_qq)
                ]

            attention_stream(0, 0, QB, 4, 0, qkv_chunks(1))
            attention_stream(1, QB, QB, 8, 0,
                             op_chunks(0, 0, 0, 0, QB) + qkv_chunks(2))
            attention_stream(2, 2 * QB, QB, 12,
                             0,
                             op_chunks(1, 0, QB, P, QB)
                             + [lambda: emit_qk_half(3, "q", 0),
                                lambda: emit_qk_half(3, "q", 1)])
            attention_stream(3, 3 * QB, 256, 14, 0,
                             op_chunks(2, 0, 2 * QB, 2 * P, QB,
                                       copy_after=(0, P))
                             + [lambda: emit_qk_half(3, "k", 0),
                                lambda: emit_qk_half(3, "k", 1)]
                             + [lambda jt=jt: emit_v_tile(jt)
                                for jt in range(12, 16)],
                             sched=[1, 2, 3, 4, 5, 6, 8, 9, 10, 11])
            attention_stream(3, 3 * QB + 256, 256, 16, 256,
                             op_chunks(3, 0, 3 * QB, 3 * P, 256,
                                       copy_after=(P, P)),
                             sched=[2, 4])
            # final half out-projection + RS (kept as ONE collective: each
            # RS pays its own peer-sync on the serial CC stream, so
            # splitting it loses more to skew than the overlap gains)
            for ch in op_chunks(3, 256, 3 * QB + 256, 3 * P + 64, 256):
                ch()
            # all copies AFTER the last RS trigger: a copy waiting on a slow
            # peer's RS at the gpsimd queue head would delay later triggers
            # (peer skew would couple into the local RS pipeline)
            for r0, nr in ((0, P), (P, P), (2 * P, P), (3 * P, 64),
                           (3 * P + 64, 64)):
                nc.gpsimd.dma_start(out_d.ap()[r0:r0 + nr, :],
                                    rs_out.ap()[r0:r0 + nr, :])

    nc.compile()
    return nc


def _get_nc():
    if "nc" not in _NC_CACHE:
        _NC_CACHE["nc"] = build_kernel()
    return _NC_CACHE["nc"]


def make_in_maps(normalized_resid_pre, W_Q, W_K, W_V, W_O, b_Q, b_K, b_V, b_O):
    x = np.asarray(normalized_resid_pre, dtype=F32)
    W_Q = np.asarray(W_Q, F32); W_K = np.asarray(W_K, F32)
    W_V = np.asarray(W_V, F32); W_O = np.asarray(W_O, F32)
    b_Q = np.asarray(b_Q, F32); b_K = np.asarray(b_K, F32)
    b_V = np.asarray(b_V, F32); b_O = np.asarray(b_O, F32)

    sq = 1.0 / (D * np.sqrt(DH))            # folded into W_Q / b_Q
    so = 1.0 / (np.sqrt(D) * np.sqrt(DH))   # folded into W_O

    wo_s = (W_O * so).reshape(H, DH, D)
    tri = np.triu(np.ones((P, P), dtype=F32)).astype(BF16)  # tri[k,q]=1 iff k<=q
    bo_b = (b_O / 4.0).reshape(1, D).astype(F32)

    in_maps = []
    for c in CORE_IDS:
        b = c // 4
        hg = c % 4
        hs = slice(HC * hg, HC * (hg + 1))
        xT_b = np.ascontiguousarray(x[b].T.astype(BF16))          # [D, S]
        wq_c = np.ascontiguousarray(
            (W_Q[hs] * sq).transpose(1, 0, 2).reshape(D, E).astype(BF16))
        wk_c = np.ascontiguousarray(
            W_K[hs].transpose(1, 0, 2).reshape(D, E).astype(BF16))
        wv_c = np.ascontiguousarray(
            W_V[hs].transpose(1, 0, 2).reshape(D, E).astype(BF16))
        wo_c = np.ascontiguousarray(
            wo_s[hs].reshape(E, D).astype(BF16))
        bq_c = np.ascontiguousarray(
            (b_Q[hs] * sq).reshape(E).reshape(2, P).T).astype(F32)  # [P, 2]
        bk_c = np.ascontiguousarray(
            b_K[hs].reshape(E).reshape(2, P).T).astype(F32)
        bv_c = np.ascontiguousarray(
            np.broadcast_to(b_V[hs].reshape(E), (P, E))).astype(F32)
        in_maps.append({
            "xT": xT_b, "wq": wq_c, "wk": wk_c, "wv": wv_c, "wo": wo_c,
            "bq": bq_c, "bk": bk_c, "bv": bv_c, "bo": bo_b, "tri": tri,
        })
    return in_maps


def assemble_out(results):
    out = np.empty((B, S, D), dtype=F32)
    for c in CORE_IDS:
        b, g = c // 4, c % 4
        r = results[c]["out"].astype(F32)
        for j in range(3):
            out[b, QB * j + P * g:QB * j + P * (g + 1), :] = \
                r[P * j:P * (j + 1)]
        out[b, 3 * QB + 64 * g:3 * QB + 64 * (g + 1), :] = r[3 * P:3 * P + 64]
        out[b, 3 * QB + 256 + 64 * g:3 * QB + 256 + 64 * (g + 1), :] = \
            r[3 * P + 64:QB]
    return out


def _ensure_trace_support():
    """If profiling is requested, make sure the axon NTFF hook shim exists
    (this container's antenv package lacks axon_hooks)."""
    try:
        import types
        import antenv

        if "antenv.axon_hooks" not in sys.modules:
            mod = types.ModuleType("antenv.axon_hooks")
            hook = [None]
            mod.set_axon_ntff_profile_hook = lambda h: hook.__setitem__(0, h)
            mod.get_axon_ntff_profile_hook = lambda: hook[0]
            sys.modules["antenv.axon_hooks"] = mod
            antenv.axon_hooks = mod
            from trn_agent_boot.trn_boot import _ntff_profile_via_ctypes

            mod.set_axon_ntff_profile_hook(
                _ntff_profile_via_ctypes("/opt/axon/libaxon_pjrt.so"))
    except Exception:
        pass


def kernel(**inputs):
    from concourse.bass_utils import run_bass_kernel_spmd

    _ensure_trace_support()
    nc = _get_nc()
    in_maps = make_in_maps(**inputs)
    trace = bool(int(os.environ.get("BASS_KERNEL_TRACE", "0")))
    res = run_bass_kernel_spmd(nc, in_maps, CORE_IDS, trace=trace)
    _NC_CACHE["last_result"] = res
    return assemble_out(res.results)

